# revision 1
# baseline (speedup 1.0000x reference)
# Trainium2 Bass kernel for nn_AttentionLayer (BiDAF-style attention).
#
# Math (T=16384, J=1024, D2=512):
#   w1,w2,w3 = Ws blocks;  S[t,j] = H@w1 + U@w2 + (H*w3)@U.T
#   A  = softmax_j(S) @ U                      (C2Q)
#   b  = softmax_t(max_j S);  h~ = b @ H       (Q2C, global over T)
#   G  = [H | A | H*A | H*h~]                  (T, 2048)
#
# Sharding: T rows split across 8 cores (2048 rows each). U/Ws replicated.
# Per core everything is local except (hnum = sum_t exp(m_t) H_t, ssum =
# sum_t exp(m_t)), exchanged via a single AllGather (one ring pass, ~7
# hops) + local sum -- measurably faster than AllReduce (reduce-scatter
# + all-gather = two ring passes) since the tiny 2KB payload makes the
# collective pure hop latency.  A dummy AllGather on a NEFF-const input
# fires at kernel start to absorb the first-collective warmup/barrier.
# NOTE the collective output must be shaped [1, 8, 520] (not [8, 520]):
# the gather-landing DMA into a single SBUF partition otherwise clobbers
# neighboring persistent tiles on partitions 1-7 on real hardware (the
# simulator normalizes the access pattern and hides it).
# Fleet-state caveat: the first-collective barrier (13-68us) and the real
# ring time (6-40us) are strongly correlated run-to-run -- a throttled
# peer core slows both.  Fast fleet => ~136us total, slow => ~155us.
#
# Layout trick: compute S^T tiles [j_part, t_free] so the C2Q attend matmul
# (A = P @ U) can use E=exp(S^T) slices directly as the stationary operand.
# exp bias handles the s2[j] term (per-partition); the s1[t] term cancels in
# softmax_j and is reapplied only to the Q2C row maxima.
#
# Perf structure (PE is the bottleneck engine, ~100us of matmul work;
# bf16 and f32r both run 1 column/cycle so bf16 only helps DMA bytes
# and LDWEIGHTS):
#  - all matmul operand pairs in bf16 (S: uw3 x ht, attend: e x un, s1:
#    wcol x ht); f32r operands must never mix with 16-bit ones (NCC_IBIR034)
#  - S and A accumulations run as two interleaved PSUM chains: same-bank
#    accumulation serializes the PE (measured 464 -> 300ns per LDW+MM pair)
#  - per chunk: phase 1 = S matmuls + exp; phase 2a = Q2C-critical
#    reductions ONLY (pmax -> emax -> bnum -> hnum feed the collective
#    trigger); phase 2b = softmax denominator (only the deferred attends
#    need it, so chunk 3's runs AFTER the trigger -- measured trigger
#    pull-in ~89us -> 83us); phase 3 = A matmuls + G writes, all
#    deferred past the trigger so the collective latency is covered.
#    Inlining any phase 3 before the trigger measured worse (156us).
#  - G block 0 (= H) is written back during the load phase: spreads the
#    16MB of G writes and thins DMA traffic under the collective ring.
#  - pmax chain all-bf16 (DVE 2x mode; bf16 max is exact), A-scaling on
#    the Scalar engine (activation Copy + per-partition scale).
#  - tail: un-normalized gathered hnum broadcast on PE in parallel with
#    the 1/ssum reciprocal (normalization folded into the PSUM->SBUF copy
#    as a partition-broadcast scale); H*h~ split GpSimd(chunk 0, own
#    tile)/DVE(chunks 1-3), writes issued in completion order c1,c2,c0,c3.
# Tile readiness is tracked PER-TILE, not per-slice: splitting the
# 2-writer `un` tile into two single-writer tiles measured 145 -> 138us.
# BUT the same split applied to `ht` (7 writers) and `hn` (4 writers)
# measured WORSE (157 vs 138 at equal fleet state) -- the early PE gap is
# not ht-gated, and the extra per-tile semaphore waits on 128+ matmuls
# cost more than they save.  Split multi-writer tiles only when a reader
# provably stalls on an unrelated writer DMA.
# UNRESOLVED lead: letting each chunk's last S jq-group borrow the two
# S-phase-idle apsum banks (5 in-flight chains; targets the ~3.5us PE gap
# at ~18-21us from exp-recycle pressure + the Scalar engine's late start)
# was correct on HW both times it ran and narrowed the gap to 2.9us, but
# both samples landed on dying fleets (94us and 103us barriers; 220/225us
# totals) -- still unverified as a win.  Re-test it FIRST on a healthy
# fleet; the edit is the pool/tag switch on the jq==6 S group.
# Known-bad variants (measured): vector.tensor_tensor_reduce hard-crashes
# the exec unit (NRT_EXEC_UNIT_UNRECOVERABLE); gpsimd tensor_tensor is
# ~3x slower than DVE (1.7us per 128x512 tile) so giving it more than 1
# tile per chunk delays the final writes; inlining phase 3 for early
# chunks (156us) and splitting S matmuls into chunk-pair chains with PE
# d-row sums (168-171us) both lose to this schedule.

import numpy as np

T, J, D2 = 16384, 1024, 512
NCORES = 8
TC = T // NCORES            # 2048 context rows per core
NCHUNK = 4                  # t-chunks per core
CHUNK = TC // NCHUNK        # 512
NTT = TC // 128             # 16 t-tiles per core
NJT = J // 128              # 8 j-tiles
NKT = D2 // 128             # 4 d-tiles

MM_BF16 = True             # bf16 for the S matmul operands (uw3, ht)
E_BF16 = True               # bf16 for E=exp(S) and U in the attend matmul

_CACHE = {}
LAST = {}


def _build_nc():
    import concourse.bacc as bacc
    import concourse.mybir as mybir
    import concourse.tile as tile

    f32 = mybir.dt.float32
    f32r = mybir.dt.float32r
    bf16 = mybir.dt.bfloat16
    mmdt = bf16 if MM_BF16 else f32r
    edt = bf16 if E_BF16 else f32r
    X = mybir.AxisListType.X
    MAX = mybir.AluOpType.max
    ADD = mybir.AluOpType.add
    MULT = mybir.AluOpType.mult
    BYP = mybir.AluOpType.bypass
    EXP = mybir.ActivationFunctionType.Exp
    CPY = mybir.ActivationFunctionType.Copy

    def f(ap):  # view an fp32r AP as plain fp32 for non-matmul consumers
        return ap.bitcast(f32) if ap.dtype == f32r else ap

    nc = bacc.Bacc("TRN2", target_bir_lowering=False, debug=False,
                   num_devices=NCORES)

    HT = nc.dram_tensor("HT", [D2, TC], mmdt, kind="ExternalInput")
    Hn = nc.dram_tensor("Hn", [TC, D2], f32r, kind="ExternalInput")
    Un = nc.dram_tensor("Un", [J, D2], edt, kind="ExternalInput")
    UW = nc.dram_tensor("UW", [D2, J], mmdt, kind="ExternalInput")
    Wc = nc.dram_tensor("Wc", [D2, 2], mmdt, kind="ExternalInput")
    W2b = nc.dram_tensor("W2b", [128, D2], bf16, kind="ExternalInput")
    Id = nc.dram_tensor("Id", [128, 128], f32, kind="ExternalInput")
    Ib = nc.dram_tensor("Ib", [128, 128], bf16, kind="ExternalInput")
    On = nc.dram_tensor("On", [1, 128], f32r, kind="ExternalInput")
    Oc = nc.dram_tensor("Oc", [128, 2], f32r, kind="ExternalInput")
    G = nc.dram_tensor("G", [TC, 4 * D2], f32, kind="ExternalOutput")

    with tile.TileContext(nc) as tc:
        with (
            tc.tile_pool(name="persist", bufs=1) as pp,
            tc.tile_pool(name="stream", bufs=2) as sp,
            tc.tile_pool(name="stage", bufs=4) as gp,
            tc.tile_pool(name="hhpool", bufs=3) as hp,
            tc.tile_pool(name="epool", bufs=4) as ep,
            tc.tile_pool(name="spsum", bufs=3, space="PSUM") as spsum,
            tc.tile_pool(name="apsum", bufs=2, space="PSUM") as apsum,
            tc.tile_pool(name="trpsum", bufs=1, space="PSUM") as trpsum,
            tc.tile_pool(name="rowpsum", bufs=1, space="PSUM") as rowpsum,
            tc.tile_pool(name="dram", bufs=1, space="DRAM") as dram,
        ):
            # ---- dummy collective first: pays the first-collective
            # warmup/barrier while the engines do real work.  Its input is a
            # NEFF-const DRAM tensor (loaded at model-load time), so the
            # trigger has zero kernel-time dependencies.
            dummy_in = nc.inline_tensor(np.zeros((1, 520), np.float32),
                                        name="dummy_in")
            dummy_out = dram.tile([1, NCORES, 520], f32, tag="dummy_out")
            nc.gpsimd.collective_compute(
                "AllGather", BYP, replica_groups=[list(range(NCORES))],
                ins=[dummy_in.ap()], outs=[dummy_out.opt()],
            )

            # ---- loads, in the order the pipeline consumes them:
            # S matmuls need uw3 + ht[chunk0]; the first exps need s2col,
            # which needs un + w2b.
            uw3 = pp.tile([128, NKT, J], mmdt, tag="uw3")
            ht = pp.tile([128, NKT, TC], mmdt, tag="ht")
            for kt in range(NKT):
                nc.sync.dma_start(
                    uw3[:, kt, :], UW.ap()[kt * 128:(kt + 1) * 128, :])
                nc.sync.dma_start(
                    ht[:, kt, 0:CHUNK],
                    HT.ap()[kt * 128:(kt + 1) * 128, 0:CHUNK])
            # un split into two single-writer tiles: a slice reader is
            # otherwise gated on BOTH half-loads, stalling the first exps
            # (s2 bias) ~3us behind the second un DMA
            una = pp.tile([128, 4, D2], edt, tag="una")
            unb = pp.tile([128, 4, D2], edt, tag="unb")

            def unt(jt):
                return una[:, jt, :] if jt < 4 else unb[:, jt - 4, :]

            w2b = pp.tile([128, D2], bf16, tag="w2b")
            nc.sync.dma_start(
                una[:],
                Un.ap()[0:512, :].rearrange("(jt p) d -> p jt d", p=128))
            nc.sync.dma_start(w2b[:], W2b.ap()[:])
            # NOTE: loading chunk-1 ht ahead of un[4:8] measured WORSE both
            # times it was tried (153-155 vs 145us): the delayed second un
            # half pushes the s2 biases for j-tiles 4-7, stalling the exps
            # of every chunk.  Keep un complete before any later ht chunk.
            nc.sync.dma_start(
                unb[:],
                Un.ap()[512:1024, :].rearrange("(jt p) d -> p jt d", p=128))
            wcol = pp.tile([128, NKT, 2], mmdt, tag="wcol")
            nc.sync.dma_start(wcol[:], Wc.ap().rearrange("(kt p) w -> p kt w", p=128))
            ident = pp.tile([128, 128], f32, tag="ident")
            nc.sync.dma_start(ident[:], Id.ap()[:])
            identb = pp.tile([128, 128], bf16, tag="identb")
            nc.sync.dma_start(identb[:], Ib.ap()[:])
            for c in range(1, NCHUNK):
                cs, ce = c * CHUNK, (c + 1) * CHUNK
                nc.sync.dma_start(
                    ht[:, :, cs:ce],
                    HT.ap()[:, cs:ce].rearrange("(kt p) t -> p kt t", p=128))
            hn = pp.tile([128, NTT, D2], f32r, tag="hn")
            for c in range(NCHUNK):
                cs, ce = c * CHUNK, (c + 1) * CHUNK
                nc.sync.dma_start(
                    hn[:, 4 * c:4 * (c + 1), :],
                    Hn.ap()[cs:ce, :].rearrange("(tt p) d -> p tt d", p=128))
                # G block 0 (= H) written back immediately: spreads the G
                # writes into the load/S phase and thins out the DMA fabric
                # during the AllReduce ring + deferred attend phase.
                nc.sync.dma_start(
                    G.ap()[cs:ce, 0:D2].rearrange("(q p) d -> p q d", p=128),
                    f(hn[:, 4 * c:4 * (c + 1), :]))
            onesrow = pp.tile([1, 128], f32r, tag="onesrow")
            nc.sync.dma_start(onesrow[:], On.ap()[:])
            onescol = pp.tile([128, 2], f32r, tag="onescol")
            nc.sync.dma_start(onescol[:], Oc.ap()[:])

            # ---- s2[j] = U @ w2 on DVE: per-(j)-partition columns directly
            # all-bf16 multiply (DVE 2x mode), f32 accumulation in the reduce
            s2col = pp.tile([128, NJT], f32, tag="s2col")
            for jt in range(NJT):
                scr = gp.tile([128, D2], bf16, tag="ttscr")
                nc.vector.tensor_tensor(scr[:], unt(jt), w2b[:], MULT)
                nc.vector.tensor_reduce(s2col[:, jt:jt + 1], scr[:], X, ADD)

            # ---- persistent accumulators
            emax = pp.tile([128, NTT], f32, tag="emax")    # max_j E'' per t
            dcol = pp.tile([128, NTT], f32, tag="dcol")    # sum_j E'' per t
            es1 = pp.tile([128, NTT], f32, tag="es1")      # exp(s1[t])
            bnum = pp.tile([128, NTT], f32r, tag="bnum")   # exp(m[t])
            hnum_sb = pp.tile([1, D2], f32, tag="hnum_sb")  # sum_t bnum*H

            # collective payload row, zero-padded up front (off trigger path)
            arow = pp.tile([1, 520], f32, tag="arow")
            nc.vector.memset(arow[:], 0.0)

            def q2c_trigger():
                # ssum = sum_t bnum[t];  exchange [hnum | ssum]
                ssps = rowpsum.tile([1, NTT], f32, tag="row", name="ssps")
                nc.tensor.matmul(ssps[:], onescol[:, 0:1], bnum[:],
                                 start=True, stop=True)
                nc.vector.tensor_copy(arow[0:1, 0:D2], hnum_sb[:])
                nc.vector.tensor_reduce(arow[0:1, D2:D2 + 1], ssps[:], X, ADD)
                # AllGather + local sum instead of AllReduce: one ring pass
                # (7 hops) instead of reduce-scatter + all-gather (14), and
                # the hops are what's slow under concurrent G-write DMA.
                ar_in = dram.tile([1, 520], f32, tag="ar_in")
                ar_out = dram.tile([1, NCORES, 520], f32, tag="ar_out")
                nc.sync.dma_start(ar_in[:], arow[:])
                nc.gpsimd.collective_compute(
                    "AllGather", BYP, replica_groups=[list(range(NCORES))],
                    ins=[ar_in.opt()], outs=[ar_out.opt()],
                )
                return ar_out

            def q2c_gather_sum(ar_out):
                # Emitted between the two deferred attend phases: the DVE
                # reaches the gather-sum as soon as chunk 2's H*A ops are
                # done, while the AllGather result is already in DRAM.
                hg8 = pp.tile([1, NCORES, 520], f32, tag="hg8")
                nc.scalar.dma_start(hg8[0:1, :, :], ar_out[:])
                # local sum of the 8 gathered partial rows (f32r tile so the
                # broadcast matmul below can consume it directly)
                hg = pp.tile([1, 520], f32r, tag="hg")
                nc.vector.tensor_tensor(hg[:], hg8[0:1, 0, :],
                                        hg8[0:1, 1, :], ADD)
                for k in range(2, NCORES):
                    nc.vector.tensor_tensor(hg[:], f(hg[:]),
                                            hg8[0:1, k, :], ADD)
                zinv = pp.tile([1, 1], f32, tag="zinv")
                nc.vector.reciprocal(zinv[:], f(hg)[0:1, D2:D2 + 1])
                zcol = pp.tile([128, 1], f32, tag="zcol")
                nc.gpsimd.partition_broadcast(zcol[:], zinv[:])
                return hg, zcol

            def q2c_finish(hg, zcol):
                # PE broadcast of the un-normalized hnum row (queued after
                # the last attend matmuls); normalization is folded into the
                # PSUM->SBUF copy as a partition-broadcast scale.
                htps = apsum.tile([128, D2], f32, tag="aps", name="htps")
                nc.tensor.matmul(htps[:], onesrow[:], hg[0:1, 0:D2],
                                 start=True, stop=True)
                hts = pp.tile([128, D2], f32, tag="hts")
                nc.scalar.activation(hts[:], htps[:], CPY, scale=zcol[:])
                return hts

            def phase3(c, e):
                # C2Q attend + G blocks 0..2, two interleaved PSUM chains
                for ip in range(0, 4, 2):
                    apss = [apsum.tile([128, D2], f32, tag="aps",
                                       name=f"aps_{c}_{ip}_{q}")
                            for q in range(2)]
                    for jt in range(NJT):
                        for q in range(2):
                            i = ip + q
                            nc.tensor.matmul(
                                apss[q][:],
                                e[:, jt, i * 128:(i + 1) * 128],
                                unt(jt),
                                start=(jt == 0), stop=(jt == NJT - 1))
                    for q in range(2):
                        i = ip + q
                        tt = 4 * c + i
                        dinv = gp.tile([128, 1], f32, tag="dinv")
                        nc.vector.reciprocal(dinv[:], dcol[:, tt:tt + 1])
                        # A = psum * (1/d) on the Scalar engine (frees DVE).
                        # NOTE: staging [A | H*A] in one tile with a single
                        # combined DMA measured WORSE (154 vs 145us at equal
                        # fleet state): it delays the A bytes behind the H*A
                        # compute, and Sync issue cost was not the gate.
                        a_sb = gp.tile([128, D2], f32, tag="a_sb")
                        nc.scalar.activation(a_sb[:], apss[q][:], CPY,
                                             scale=dinv[:])
                        ha_sb = gp.tile([128, D2], f32, tag="ha_sb")
                        nc.vector.tensor_tensor(ha_sb[:], f(hn[:, tt, :]),
                                                a_sb[:], MULT)
                        ts_, te_ = tt * 128, (tt + 1) * 128
                        nc.sync.dma_start(G.ap()[ts_:te_, D2:2 * D2], a_sb[:])
                        nc.sync.dma_start(G.ap()[ts_:te_, 2 * D2:3 * D2],
                                          ha_sb[:])

            hts = None
            deferred = []
            for c in range(NCHUNK):
                cs, ce = c * CHUNK, (c + 1) * CHUNK

                # ---- phase 1: S^T tiles -> E'' = exp(S^T + s2[j]),
                # two interleaved PSUM chains
                e = ep.tile([128, NJT, CHUNK], edt, tag="e")
                for jq in range(0, NJT, 2):
                    spss = [spsum.tile([128, CHUNK], f32, tag="sps",
                                       name=f"sps_{c}_{jq}_{q}")
                            for q in range(2)]
                    for kt in range(NKT):
                        for q in range(2):
                            nc.tensor.matmul(
                                spss[q][:],
                                uw3[:, kt, (jq + q) * 128:(jq + q + 1) * 128],
                                ht[:, kt, cs:ce],
                                start=(kt == 0), stop=(kt == NKT - 1))
                    for q in range(2):
                        nc.scalar.activation(e[:, jq + q, :], spss[q][:], EXP,
                                             bias=s2col[:, jq + q:jq + q + 1])

                # ---- phase 2a: Q2C-critical reductions ONLY (the
                # pmax->emax->bnum->hnum chain feeds the collective trigger;
                # the softmax denominator does not -- it moves to phase 2b).
                # pmax runs all-bf16 (DVE 2x mode; max of bf16 is exact).
                pmax = sp.tile([128, CHUNK], bf16, tag="pmax")
                nc.vector.tensor_tensor(pmax[:], e[:, 0, :], e[:, 1, :], MAX)
                for jt in range(2, NJT):
                    nc.vector.tensor_tensor(pmax[:], pmax[:], e[:, jt, :], MAX)

                # s1[t] rows via PE (w1 column stationary), then transpose
                s1ps = rowpsum.tile([1, CHUNK], f32, tag="row",
                                    name=f"s1ps_{c}")
                for kt in range(NKT):
                    nc.tensor.matmul(s1ps[:], wcol[:, kt, 0:1],
                                     ht[:, kt, cs:ce],
                                     start=(kt == 0), stop=(kt == NKT - 1))
                s1row = sp.tile([1, CHUNK], bf16, tag="s1row")
                nc.vector.tensor_copy(s1row[:], s1ps[:])

                # batched per-engine emission: all transposes, then the
                # reduce/exp/mult stages, then the hnum chain.  The previous
                # per-tile interleave made the PE queue block ~1us per tile
                # on the 6-hop PE->DVE->PE->Scalar->DVE->PE round-trip.
                hnps = rowpsum.tile([1, D2], f32, tag="row", name=f"hnps_{c}")
                # one combined bf16 PSUM tile: 4 pmax transposes + 4 s1
                # columns, so the whole batch shares one pool tag and never
                # couples to the denominator (dcol) tag's rotation
                trc = trpsum.tile([128, 4, 130], bf16, tag="tr",
                                  name=f"trc_{c}")
                for i in range(4):
                    nc.tensor.transpose(trc[:, i, 0:128],
                                        pmax[:, i * 128:(i + 1) * 128],
                                        identb[:])
                    nc.tensor.transpose(trc[:, i, 128:129],
                                        s1row[0:1, i * 128:(i + 1) * 128],
                                        identb[0:1, 0:1])
                for i in range(4):
                    tt = 4 * c + i
                    nc.vector.tensor_reduce(emax[:, tt:tt + 1],
                                            trc[:, i, 0:128], X, MAX)
                    nc.scalar.activation(es1[:, tt:tt + 1],
                                         trc[:, i, 128:129], EXP)
                for i in range(4):
                    tt = 4 * c + i
                    # bnum = exp(m[t]) = emax * exp(s1)
                    nc.vector.tensor_tensor(bnum[:, tt:tt + 1],
                                            emax[:, tt:tt + 1],
                                            es1[:, tt:tt + 1], MULT)
                for i in range(4):
                    tt = 4 * c + i
                    # Q2C numerator: hnps += bnum_tile.T @ H_tile
                    nc.tensor.matmul(hnps[:], bnum[:, tt:tt + 1],
                                     hn[:, tt, :],
                                     start=(i == 0), stop=(i == 3))
                if c == 0:
                    nc.vector.tensor_copy(hnum_sb[:], hnps[:])
                else:
                    nc.vector.tensor_tensor(hnum_sb[:], hnum_sb[:], hnps[:], ADD)

                # ---- phase 2b: softmax denominator (needed only by the
                # deferred attends, so chunk 3's runs AFTER the collective
                # trigger -- pulls the trigger ~5us earlier)
                def phase2d(cd, ed):
                    psm = sp.tile([128, CHUNK], f32r, tag="psm",
                                  name=f"psm_{cd}")
                    nc.vector.tensor_tensor(psm[:], f(ed[:, 0, :]),
                                            f(ed[:, 1, :]), ADD)
                    for jt in range(2, NJT):
                        nc.vector.tensor_tensor(psm[:], f(psm[:]),
                                                f(ed[:, jt, :]), ADD)
                    for i in range(4):
                        tt = 4 * cd + i
                        dps = trpsum.tile([128, 2], f32, tag="dcol",
                                          name=f"dps_{cd}_{i}")
                        nc.tensor.matmul(dps[:],
                                         psm[:, i * 128:(i + 1) * 128],
                                         onescol[:], start=True, stop=True)
                        nc.vector.tensor_copy(dcol[:, tt:tt + 1], dps[:, 0:1])

                # all attend phases are deferred until after the
                # collective trigger so its latency is fully covered by
                # the attend matmuls + G writes
                deferred.append((c, e))
                if c < NCHUNK - 1:
                    phase2d(c, e)
                else:
                    ar_out = q2c_trigger()
                    phase2d(c, e)
                    for cc, ee in deferred:
                        phase3(cc, ee)
                    hg, zcol = q2c_gather_sum(ar_out)
                    hts = q2c_finish(hg, zcol)

            # ---- G block 3: H * h~.  The 16 multiplies are the serial tail
            # after the broadcast: GpSimd (1.7us/tile) takes chunk 0 in its
            # OWN tile (sharing a tile with the DVE measured 2.5x slowdown
            # from SBUF port contention) while the DVE (0.69us/tile) does
            # chunks 1-3 in 2-tile groups.  Writes are issued from the
            # Scalar queue (idle after hts) so they bypass the Sync queue's
            # backlog of chunk-3 attend writes, in completion order.
            def g3_write(rs, ntile, tile):
                nc.scalar.dma_start(
                    G.ap()[rs:rs + ntile * 128, 3 * D2:4 * D2]
                    .rearrange("(q p) d -> p q d", p=128),
                    tile[:])

            hh0 = hp.tile([128, 4, D2], f32, tag="hh0", name="hh0")
            for i in range(4):
                nc.gpsimd.tensor_tensor(hh0[:, i, :], f(hn[:, i, :]),
                                        hts[:], MULT)
            late = []
            for cq in (1, 2, 3):
                for half in range(2):
                    hh = hp.tile([128, 2, D2], f32, tag="hh2",
                                 name=f"hh_{cq}_{half}")
                    for k in range(2):
                        tt = 4 * cq + 2 * half + k
                        nc.vector.tensor_tensor(hh[:, k, :], f(hn[:, tt, :]),
                                                hts[:], MULT)
                    if cq < 3:
                        g3_write(cq * CHUNK + half * 256, 2, hh)
                    else:
                        late.append((cq * CHUNK + half * 256, hh))
            g3_write(0, 4, hh0)
            for rs, hh in late:
                g3_write(rs, 2, hh)

    nc.compile()
    return nc


def kernel(H, U, Ws):
    import concourse.mybir as mybir
    from concourse import bass_utils

    H = np.ascontiguousarray(np.asarray(H, dtype=np.float32))
    U = np.ascontiguousarray(np.asarray(U, dtype=np.float32))
    Ws = np.asarray(Ws, dtype=np.float32)

    if "nc" not in _CACHE:
        _CACHE["nc"] = _build_nc()
    nc = _CACHE["nc"]

    mmnp = (mybir.dt.np(mybir.dt.bfloat16) if MM_BF16 else np.float32)
    ednp = (mybir.dt.np(mybir.dt.bfloat16) if E_BF16 else np.float32)

    w1 = Ws[0:D2, 0]
    w2 = Ws[D2:2 * D2, 0]
    w3 = Ws[2 * D2:3 * D2, 0]
    UW = np.ascontiguousarray(U.T * w3[:, None]).astype(mmnp)
    Unc = U.astype(ednp)
    Wc = np.ascontiguousarray(np.stack([w1, w2], axis=1)).astype(mmnp)  # [512, 2]
    W2b = np.ascontiguousarray(np.broadcast_to(w2, (128, D2))).astype(
        mybir.dt.np(mybir.dt.bfloat16))
    ident = np.eye(128, dtype=np.float32)

    in_maps = []
    for c in range(NCORES):
        Hc = H[c * TC:(c + 1) * TC]
        in_maps.append({
            "HT": np.ascontiguousarray(Hc.T).astype(mmnp),
            "Hn": Hc,
            "Un": Unc,
            "UW": UW,
            "Wc": Wc,
            "W2b": W2b,
            "Id": ident,
            "Ib": ident.astype(mybir.dt.np(mybir.dt.bfloat16)),
            "On": np.ones((1, 128), dtype=np.float32),
            "Oc": np.ones((128, 2), dtype=np.float32),
        })

    res = bass_utils.run_bass_kernel_spmd(
        nc, in_maps, core_ids=list(range(NCORES)))
    LAST["exec_time_ns"] = res.exec_time_ns
    G_full = np.concatenate([res.results[c]["G"] for c in range(NCORES)],
                            axis=0)
    return G_full.astype(np.float32, copy=False)

